# revision 22
# baseline (speedup 1.0000x reference)
"""Multi-head attention (B=8, S=1024, D=1024, H=16) on 8 TRN2 NeuronCores.

Sharding: pure data-parallel over batch — each core computes one batch
element end-to-end (weights replicated per core), so no collectives.

Per-core pipeline (projections in bf16, scores in float32r on f32
projection outputs, probabilities/V in bf16, fp32 accumulate
everywhere):
  1. Inputs cast f32->bf16 during DMA (SWDGE), PE-transposed to [D,S]
     chunk tiles (contraction dim on partitions). v first (feeds VA),
     then q,k.
  2. proj_V: V = v @ w_v + b_v, natural orientation, scattered into
     per-head "Vaug" tiles [s_block, head, 65] with a ones column.
     QT/KT projections are deferred into the attention loop (one
     128-row head-pair block each per pair) so they hide under the
     attention phase, which saturates PE+ACT.
  3. Attention per head-pair per i-half: scores^T[j,i] via K=64
     row-packed matmul pairs (f32r) -> one exp(s/8) per (jb) on ScalarE
     straight out of a 2-bank PSUM tile holding both heads (bf16 out)
     -> PV matmul u[65,i] = Vaug^T @ expT accumulated over j chunks;
     row 64 of u is sum(exp) for free (ones column).
  4. Output: PE-transpose u [65,128] slabs -> [i_part, 65], multiply by
     reciprocal of column 64 (per-partition scalar), DMA [128,64] slabs
     (strided columns) to DRAM.
"""
import numpy as np
from contextlib import ExitStack

import concourse.bass as bass
import concourse.mybir as mybir
import concourse.tile as tile
from concourse import bacc
from concourse.bass_utils import run_bass_kernel_spmd
from concourse.masks import make_identity

F32 = mybir.dt.float32
F32R = mybir.dt.float32r
BF16 = mybir.dt.bfloat16

B, S, D, H, DK = 8, 1024, 1024, 16, 64
P = 128
NB = S // P            # 8 row/col blocks
NPAIR = H // 2         # 8 head pairs (one pair per 128-row tile)
HALF = 512             # i-half width (PSUM bank = 512 fp32)
N_CORES = 8

_compiled = {}


def _build(use_bias=True):
    nc = bacc.Bacc("TRN2", target_bir_lowering=False, debug=False,
                   enable_asserts=False, num_devices=N_CORES)

    dq = nc.dram_tensor("q", [S, D], F32, kind="ExternalInput").ap()
    dk_ = nc.dram_tensor("k", [S, D], F32, kind="ExternalInput").ap()
    dv = nc.dram_tensor("v", [S, D], F32, kind="ExternalInput").ap()
    dwq = nc.dram_tensor("wq", [D, D], F32, kind="ExternalInput").ap()
    dwk = nc.dram_tensor("wk", [D, D], F32, kind="ExternalInput").ap()
    dwv = nc.dram_tensor("wv", [D, D], F32, kind="ExternalInput").ap()
    dbq = nc.dram_tensor("bq", [D], F32, kind="ExternalInput").ap()
    dbk = nc.dram_tensor("bk", [D], F32, kind="ExternalInput").ap()
    dbv = nc.dram_tensor("bv", [D], F32, kind="ExternalInput").ap()
    dout = nc.dram_tensor("out", [S, D], F32, kind="ExternalOutput").ap()

    with tile.TileContext(nc) as tc:
        with ExitStack() as ctx:
            const = ctx.enter_context(tc.tile_pool(name="const", bufs=1))
            persist = ctx.enter_context(tc.tile_pool(name="persist", bufs=1))
            ps = ctx.enter_context(tc.tile_pool(name="ps", bufs=1, space="PSUM"))

            ident = const.tile([P, P], F32)
            make_identity(nc, ident)
            ident_bf = const.tile([P, P], BF16)
            make_identity(nc, ident_bf)
            ones_f32 = const.tile([P, P], F32)
            nc.vector.memset(ones_f32[:], 1.0)
            ones_row = const.tile([1, S], F32R)
            for j in range(NB):
                nc.vector.tensor_copy(ones_row[:, j * P:(j + 1) * P],
                                      ones_f32[0:1, :])

            def load_bias(pool, dsrc, name):
                b_sb = pool.tile([1, D], F32R, name=name)
                nc.sync.dma_start(b_sb[:], dsrc[None, :].bitcast(F32R))
                return b_sb

            # persistent products of the projection phase
            QT = [persist.tile([P, S], F32R, name=f"QT{m}") for m in range(NB)]
            KT = [persist.tile([P, S], F32R, name=f"KT{m}") for m in range(NB)]
            # vaug[r]: [s_block, head, dk+1]; col 64 = ones
            VA = [persist.tile([P, H, DK + 1], BF16, name=f"VA{r}") for r in range(NB)]

            def transpose_in(pool, dsrc, name):
                """cast-load [S,D] f32->bf16, PE-transpose to chunk tiles [P(k),S]"""
                xt = [pool.tile([P, S], BF16, name=f"{name}{c}") for c in range(NB)]
                for r in range(NB):
                    natf = pool.tile([P, D], F32, name=f"natf_{name}", tag="natf",
                                     bufs=2)
                    nc.sync.dma_start(natf[:], dsrc[r * P:(r + 1) * P, :])
                    nat = pool.tile([P, D], BF16, name=f"nat_{name}", tag="natq",
                                    bufs=2)
                    nc.vector.tensor_copy(nat[:], natf[:])
                    for c in range(NB):
                        tp0 = ps.tile([P, P], BF16, name=f"tp0_{name}_{r}_{c}",
                                      tag="sc", bufs=2)
                        nc.tensor.transpose(tp0[:], nat[:, c * P:(c + 1) * P],
                                            ident_bf[:])
                        nc.vector.tensor_copy(xt[c][:, r * P:(r + 1) * P], tp0[:])
                return xt

            def proj_T_block(pool, xt, dw, b_sb, dst, m):
                """dst[m][dblock, s] = ((x @ w)^T + b)[m] : lhsT=w col chunk, rhs=xT"""
                wcol = pool.tile([P, NB, P], BF16, name="wcolT", tag="wcol", bufs=2)
                nc.gpsimd.dma_start(
                    out=wcol[:],
                    in_=dw.rearrange("(kc kp) d -> kp kc d", kp=P)
                          [:, :, m * P:(m + 1) * P])
                for hf in range(2):
                    pj = ps.tile([P, HALF], F32, name=f"pj_{m}_{hf}", tag="proj",
                                 bufs=2)
                    for kc in range(NB):
                        nc.tensor.matmul(pj[:], wcol[:, kc, :],
                                         xt[kc][:, hf * HALF:(hf + 1) * HALF],
                                         start=(kc == 0),
                                         stop=(not use_bias and kc == NB - 1))
                    if use_bias:
                        nc.tensor.matmul(pj[:], b_sb[:, m * P:(m + 1) * P],
                                         ones_row[:, hf * HALF:(hf + 1) * HALF],
                                         start=False, stop=True)
                    nc.scalar.copy(dst[m][:, hf * HALF:(hf + 1) * HALF], pj[:])

            def proj_V(pool, xt, bv_sb):
                """VA[r][s_block, h, 0:64] = (v @ w_v + b_v), natural orientation"""
                for hf in range(2):
                    wv_h = []
                    for kc in range(NB):
                        wt = pool.tile([P, HALF], BF16, name=f"WVh_{hf}_{kc}",
                                       tag=f"WVh{kc}", bufs=1)
                        nc.gpsimd.dma_start(
                            out=wt[:],
                            in_=dwv[kc * P:(kc + 1) * P,
                                    hf * HALF:(hf + 1) * HALF])
                        wv_h.append(wt)
                    for r in range(NB):
                        pj = ps.tile([P, HALF], F32, name=f"pv_{hf}_{r}", tag="proj",
                                     bufs=2)
                        for kc in range(NB):
                            nc.tensor.matmul(
                                pj[:], xt[kc][:, r * P:(r + 1) * P], wv_h[kc][:],
                                start=(kc == 0),
                                stop=(not use_bias and kc == NB - 1))
                        if use_bias:
                            nc.tensor.matmul(pj[:], ones_row[:, r * P:(r + 1) * P],
                                             bv_sb[:, hf * HALF:(hf + 1) * HALF],
                                             start=False, stop=True)
                        # scatter halves into per-head vaug slices (bf16)
                        for hh in range(8):
                            h = hf * 8 + hh
                            nc.scalar.copy(
                                VA[r][:, h, 0:DK], pj[:, hh * DK:(hh + 1) * DK])
                for r in range(NB):
                    nc.vector.memset(VA[r][:, :, DK:DK + 1], 1.0)

            def attention_pair(attn, pr):
                for hf in range(2):
                    isl = slice(hf * HALF, (hf + 1) * HALF)
                    # both heads' probs in one tile: [j_part, jb, head, i]
                    eT = attn.tile([P, NB, 2, HALF], BF16,
                                   name=f"eT{hf}_{pr}", tag="eT", bufs=2)
                    for jb in range(NB):
                        sc = ps.tile([P, 2, HALF], F32,
                                     name=f"sc_{hf}_{pr}_{jb}",
                                     tag="sc", bufs=2)
                        for hh in range(2):
                            nc.tensor.matmul(
                                sc[:, hh, :],
                                KT[pr][hh * DK:(hh + 1) * DK, jb * P:(jb + 1) * P],
                                QT[pr][hh * DK:(hh + 1) * DK, isl],
                                start=True, stop=True)
                        nc.scalar.activation(
                            out=eT[:, jb, :, :], in_=sc[:],
                            func=mybir.ActivationFunctionType.Exp,
                            scale=0.125)
                    for hh in range(2):
                        h = 2 * pr + hh
                        pu = ps.tile([DK + 1, HALF], F32,
                                     name=f"pu_{hf}_{pr}_{hh}", tag="pv", bufs=1)
                        for jb in range(NB):
                            nc.tensor.matmul(
                                pu[:], VA[jb][:, h, :],
                                eT[:, jb, hh, :],
                                start=(jb == 0), stop=(jb == NB - 1))
                        uh = attn.tile([DK + 1, HALF], F32,
                                       name=f"uh_{hf}_{pr}_{hh}", tag="uh", bufs=4)
                        nc.vector.tensor_copy(uh[:], pu[:])
                        for bl in range(HALF // P):
                            ib = hf * (HALF // P) + bl
                            tp = ps.tile([P, DK + 1], F32,
                                         name=f"tp_{hf}_{pr}_{hh}_{bl}",
                                         tag="tp", bufs=1)
                            nc.tensor.transpose(
                                tp[:], uh[:, bl * P:(bl + 1) * P],
                                ident[0:DK + 1, 0:DK + 1])
                            recip = attn.tile([P, 1], F32,
                                              name=f"rc_{hf}_{pr}_{hh}_{bl}",
                                              tag="rc", bufs=2)
                            nc.vector.reciprocal(recip[:], tp[:, DK:DK + 1])
                            opair = attn.tile([P, DK], F32,
                                              name=f"op_{hf}_{pr}_{hh}_{bl}",
                                              tag="opair", bufs=4)
                            nc.vector.tensor_scalar_mul(opair[:], tp[:, 0:DK],
                                                        recip[:])
                            nc.sync.dma_start(
                                dout[ib * P:(ib + 1) * P,
                                     h * DK:(h + 1) * DK], opair[:])

            with tc.tile_pool(name="qk_pool", bufs=1) as qk_pool:
                with tc.tile_pool(name="vt_pool", bufs=1) as vt_pool:
                    bv_sb = load_bias(vt_pool, dbv, "bv_sb")
                    vt = transpose_in(vt_pool, dv, "vt")
                    qt = transpose_in(qk_pool, dq, "qt")
                    kt = transpose_in(qk_pool, dk_, "kt")
                    proj_V(vt_pool, vt, bv_sb)
                bq_sb = load_bias(qk_pool, dbq, "bq_sb")
                bk_sb = load_bias(qk_pool, dbk, "bk_sb")
                with tc.tile_pool(name="attn", bufs=1) as attn:
                    for pr in range(NPAIR):
                        proj_T_block(qk_pool, qt, dwq, bq_sb, QT, pr)
                        proj_T_block(qk_pool, kt, dwk, bk_sb, KT, pr)
                        attention_pair(attn, pr)

    nc.compile()
    return nc


def kernel(q, k, v, w_q, b_q, w_k, b_k, w_v, b_v):
    use_bias = bool(np.any(np.asarray(b_q)) or np.any(np.asarray(b_k))
                    or np.any(np.asarray(b_v)))
    if use_bias not in _compiled:
        _compiled[use_bias] = _build(use_bias)
    nc = _compiled[use_bias]

    f = lambda x: np.ascontiguousarray(np.asarray(x, dtype=np.float32))
    in_maps = []
    for c in range(N_CORES):
        in_maps.append({
            "q": f(q[c]), "k": f(k[c]), "v": f(v[c]),
            "wq": f(w_q), "wk": f(w_k), "wv": f(w_v),
            "bq": f(b_q), "bk": f(b_k), "bv": f(b_v),
        })
    res = run_bass_kernel_spmd(nc, in_maps, list(range(N_CORES)))
    out = np.stack([res.results[c]["out"] for c in range(N_CORES)], axis=0)
    kernel.last_results = res
    return out
